# revision 16
# baseline (speedup 1.0000x reference)
"""Multi-head attention (B=2, S=2048, E=1024, H=16) on 8 Trainium2 NeuronCores.

Sharding: core c -> batch c//4, heads 4*(c%4)..4*(c%4)+3  (data + head parallel).
Each core computes a partial output projection [S, E] over its 256 head-dims;
the host sums the 4 partials per batch and adds the output bias (the
"all-reduce" happens in the unshard step).

On-chip layouts (contraction always on the partition dim, no on-chip
transposes; host pre-transposes query/key/value):
  QT, KT  [dim, S]   = Wx^T @ X^T      (rhs = X^T chunks streamed from HBM)
  V       [S, dim+ones]                 (natural; a ones column per head makes
                                         the PV matmul also emit softmax sums)
  scoresT [keys, q]  = KT_tile^T-block @ QT       per (head, q-group, key-tile)
  probsT  = exp(scoresT - 32)           (constant-shift softmax: exact up to
                                         fp32 rounding; masked lanes underflow
                                         to exactly 0 like the reference)
  attnoutT [d, q]    = (V|1)^T @ probsT (row 64 = Z = sum of probs)
  Zinv    = exp(-ln(Z))  on ACT;  broadcast across partitions via a K=1 matmul
  partial [S, E]     = attnoutT^T-chunks @ Wo-rows, accumulated over 4 heads

Matmul operands are bf16 (full PE rate); all accumulation is fp32 in PSUM.
"""

import sys

for _p in ("/opt/trn_rl_repo", "/root/.axon_site/_ro/trn_rl_repo"):
    if _p not in sys.path:
        sys.path.insert(0, _p)

import numpy as np


# ---------------------------------------------------------------------------
# Patch: the walrus build in this container rejects >1 sem wait on one CTRL
# instruction ("Too many sync wait commands") and the TileContext exit drain
# aggregates every outstanding proc's wait onto a single Drain. Spill the
# excess waits onto SP nops (1 wait each) emitted right after the drain.
# ---------------------------------------------------------------------------
def _install_tile_drain_patch():
    import concourse.tile as tile
    import concourse.mybir as mybir
    from concourse.vector_clock import ScopedClock

    if getattr(tile.TileContext, "_drain_patch_installed", False):
        return

    def _patched_drain_and_barrier(self, tick_clock, wait_clock):
        drain_inst = self.nc.sync.drain()
        wait_clock.add_sem_waits(
            drain_inst.ins, ScopedClock({None: tick_clock.global_clock})
        )
        si = drain_inst.ins.sync_info
        waits = list(si.on_wait) if si and si.on_wait else []
        if len(waits) > 1:
            si.on_wait = waits[:1]
            for w in waits[1:]:
                nop = self.nc.sync.nop(nofuse=True, hint="drain_wait_spill")
                nop.ins.sync_info = mybir.SyncInfo(on_wait=[w], on_update=[])
        self.nc.all_engine_barrier()
        assert self.sems is not None
        popped = self.nc._tile_sem_poison_stack.pop()
        assert popped is self._sem_poison
        self.nc.clear_and_free_semaphores(list(self.sems.allocated().values()))
        self.nc.all_engine_barrier()

    tile.TileContext._drain_and_barrier = _patched_drain_and_barrier
    tile.TileContext._drain_patch_installed = True


def _split_multi_waits(nc, maxw=1):
    """Walrus here allows only `maxw` sem-wait commands per instruction.
    Hoist excess waits onto engine-queue NoOps inserted just before the
    instruction (the sequencer executes them in order, so semantics are
    identical)."""
    import concourse.mybir as mybir

    ctr = 0
    for bb in nc.main_func.blocks:
        new = []
        for inst in bb.instructions:
            si = inst.sync_info
            waits = list(si.on_wait) if si and si.on_wait else []
            if len(waits) > maxw:
                extras = waits[:-maxw]
                si.on_wait = waits[-maxw:]
                for i in range(0, len(extras), maxw):
                    nop = mybir.InstNoOp(
                        name=f"I-waitspill-{ctr}", engine=inst.engine,
                        ins=[], outs=[])
                    ctr += 1
                    nop.sync_info = mybir.SyncInfo(
                        on_wait=extras[i:i + maxw], on_update=[])
                    new.append(nop)
            new.append(inst)
        bb.instructions = new


# ---------------------------------------------------------------------------
# Mask classification (host side, from the actual mask array).
# Blocks are 128x128 in the *transposed* score layout: block (kt, qb) covers
# keys kt*128.. x queries qb*128... Returns per-block bias indices into a
# stack of unique additive-bias blocks (0 where attended, -1e9 where masked).
# ---------------------------------------------------------------------------
def classify_mask(mask2d, S, KB=128):
    nb = S // KB
    assert mask2d.shape == (S, S)
    assert mask2d.any(axis=1).all(), "a query row with no attended key"
    maskT = mask2d.T  # [keys, q]
    uniq = {}
    biases = []
    bias_idx = {}  # (kt, qb) -> None (all attended) or index
    block_live = np.zeros((nb, nb), dtype=bool)  # any attended key in block
    for kt in range(nb):
        for qb in range(nb):
            blk = maskT[kt * KB:(kt + 1) * KB, qb * KB:(qb + 1) * KB]
            if blk.all():
                bias_idx[(kt, qb)] = None
                block_live[kt, qb] = True
            else:
                b = np.where(blk, np.float32(0.0), np.float32(-1e9))
                key = b.tobytes()
                if key not in uniq:
                    uniq[key] = len(biases)
                    biases.append(b)
                bias_idx[(kt, qb)] = uniq[key]
                block_live[kt, qb] = blk.any()
    return bias_idx, biases, block_live


# ---------------------------------------------------------------------------
# Bass program builder (one SPMD program, same for all cores).
# ---------------------------------------------------------------------------
def build_nc(S, E, D, HL, bias_idx, block_live, nuniq, shift=32.0):
    import concourse.bass as bass
    import concourse.mybir as mybir
    import concourse.tile as tile

    f32 = mybir.dt.float32
    bf16 = mybir.dt.bfloat16
    Act = mybir.ActivationFunctionType

    P = 128
    EC = E // P              # E chunks (contraction tiles for projections)
    DIM = HL * D             # this core's head dims (256)
    MT = DIM // P            # m-tiles of QT/KT (2)
    QG = 512                 # q-group width
    NQG = S // QG
    NKT = S // P             # key tiles
    NST = S // P             # s tiles
    VW = HL * (D + 1)        # V width incl. ones columns (260)
    EGW = min(QG, E)         # output E slice width
    NEG = E // EGW           # output E slices (2)

    # key tiles needed per q-group
    def kts_for_group(g):
        out = []
        for kt in range(NKT):
            if any(block_live[kt, g * (QG // P) + j] for j in range(QG // P)):
                out.append(kt)
        return out

    nc = bass.Bass()
    dp = nc.declare_dram_parameter
    d_xq = dp("xqT", [E, S], bf16, isOutput=False)
    d_xk = dp("xkT", [E, S], bf16, isOutput=False)
    d_xv = dp("xvT", [E, S], bf16, isOutput=False)
    d_wq = dp("wq", [E, DIM], bf16, isOutput=False)
    d_wk = dp("wk", [E, DIM], bf16, isOutput=False)
    d_wv = dp("wv", [E, VW], bf16, isOutput=False)
    d_wo = dp("wo", [DIM, E], bf16, isOutput=False)
    d_bias = dp("biasT", [P, max(nuniq, 1) * P], f32, isOutput=False)
    d_out = dp("out_p", [S, E], f32, isOutput=True)

    import contextlib
    with tile.TileContext(nc) as tc, contextlib.ExitStack() as _stk:
        consts = _stk.enter_context(tc.tile_pool(name="consts", bufs=1))

        # weights: [E, n] rearranged so one DMA loads all chunks:
        # chunk e lives at w_sb[:, e, :]
        w_sb = {}
        for nm, dram, width in (("wq", d_wq, DIM), ("wk", d_wk, DIM),
                                ("wv", d_wv, VW)):
            t = consts.tile([P, EC, width], bf16, name=f"sb_{nm}", tag=f"sb_{nm}")
            nc.sync.dma_start(
                out=t, in_=dram[:, :].rearrange("(e p) n -> p e n", p=P))
            w_sb[nm] = t
        # wo per head [D, E] (partitions 0..D)
        wo_sb = []
        for h in range(HL):
            t = consts.tile([D, E], bf16, name=f"sb_wo{h}", tag=f"sb_wo{h}")
            nc.sync.dma_start(out=t, in_=d_wo[h * D:(h + 1) * D, :])
            wo_sb.append(t)
        bias_sb = consts.tile([P, max(nuniq, 1) * P], f32, name="sb_bias")
        nc.sync.dma_start(out=bias_sb, in_=d_bias[:, :])
        ones64 = consts.tile([1, D], bf16, name="ones64")
        nc.vector.memset(ones64, 1.0)
        negshift = consts.tile([P, 1], f32, name="negshift")
        nc.vector.memset(negshift, -shift)

        # persistent projection outputs
        QT = [consts.tile([P, S], bf16, name=f"QT{m}", tag=f"QT{m}")
              for m in range(MT)]
        KT = [consts.tile([P, S], bf16, name=f"KT{m}", tag=f"KT{m}")
              for m in range(MT)]
        V = [consts.tile([P, VW], bf16, name=f"V{s}", tag=f"V{s}")
             for s in range(NST)]

        # ---------------- phase A: projections ----------------
        with tc.tile_pool(name="xt", bufs=2 * EC + 2) as xt_pool, \
             tc.tile_pool(name="psA", bufs=4, space="PSUM") as psA:

            def stream_chunks(dram):
                chunks = []
                for e in range(EC):
                    ch = xt_pool.tile([P, S], bf16, tag="xt", name=f"xch{e}")
                    nc.sync.dma_start(out=ch, in_=dram[e * P:(e + 1) * P, :])
                    chunks.append(ch)
                return chunks

            # QT / KT:  out^T [dim, S] = sum_e W[e-chunk, m-tile]^T-block
            for wname, dst in (("wq", QT), ("wk", KT)):
                chunks = stream_chunks(d_xq if wname == "wq" else d_xk)
                for m in range(MT):
                    for g in range(NQG):
                        ps = psA.tile([P, QG], f32, tag="psA")
                        for e in range(EC):
                            nc.tensor.matmul(
                                ps,
                                lhsT=w_sb[wname][:, e, m * P:(m + 1) * P],
                                rhs=chunks[e][:, g * QG:(g + 1) * QG],
                                start=(e == 0), stop=(e == EC - 1))
                        nc.vector.tensor_copy(dst[m][:, g * QG:(g + 1) * QG], ps)

            # V natural [S, VW] = X chunks @ Wv
            chunks = stream_chunks(d_xv)
            for st in range(NST):
                ps = psA.tile([P, VW], f32, tag="psA")
                for e in range(EC):
                    nc.tensor.matmul(
                        ps,
                        lhsT=chunks[e][:, st * P:(st + 1) * P],
                        rhs=w_sb["wv"][:, e, :],
                        start=(e == 0), stop=(e == EC - 1))
                nc.vector.tensor_copy(V[st], ps)
                # ones columns (Wv has zero columns there; overwrite with 1.0)
                onescols = V[st].rearrange("p (h c) -> p h c", c=D + 1)[:, :, D]
                nc.vector.memset(onescols, 1.0)

        # ---------------- phase B: attention ----------------
        attnT = [consts.tile([D, S], bf16, name=f"attnT{h}", tag=f"attnT{h}")
                 for h in range(HL)]

        with tc.tile_pool(name="probs", bufs=3) as probs_pool, \
             tc.tile_pool(name="evB", bufs=2) as evB_pool, \
             tc.tile_pool(name="zrow", bufs=2) as z_pool, \
             tc.tile_pool(name="psS", bufs=3, space="PSUM") as psS, \
             tc.tile_pool(name="psPV", bufs=2, space="PSUM") as psPV, \
             tc.tile_pool(name="psB", bufs=2, space="PSUM") as psB:

            for h in range(HL):
                m, po = h // 2, (h % 2) * D
                ev = evB_pool.tile([D + 1, S], f32, tag="ev", name=f"ev{h}")
                for g in range(NQG):
                    kts = kts_for_group(g)
                    pv = psPV.tile([D + 1, QG], f32, tag="pv")
                    for i, kt in enumerate(kts):
                        sps = psS.tile([P, QG], f32, tag="s")
                        nc.tensor.matmul(
                            sps,
                            lhsT=KT[m][po:po + D, kt * P:(kt + 1) * P],
                            rhs=QT[m][po:po + D, g * QG:(g + 1) * QG],
                            start=True, stop=True)
                        for j in range(QG // P):
                            bidx = bias_idx[(kt, g * (QG // P) + j)]
                            if bidx is not None:
                                nc.vector.tensor_add(
                                    sps[:, j * P:(j + 1) * P],
                                    sps[:, j * P:(j + 1) * P],
                                    bias_sb[:, bidx * P:(bidx + 1) * P])
                        pb = probs_pool.tile([P, QG], bf16, tag="pb")
                        nc.scalar.activation(pb, sps, Act.Exp,
                                             bias=negshift[:, 0:1])
                        nc.tensor.matmul(
                            pv,
                            lhsT=V[kt][:, h * (D + 1):(h + 1) * (D + 1)],
                            rhs=pb,
                            start=(i == 0), stop=(i == len(kts) - 1))
                    nc.vector.tensor_copy(ev[:, g * QG:(g + 1) * QG], pv)

                # Zinv = exp(-ln(Z)) for the whole head at once (both fns are
                # in the natural_log_exp_and_others ACT table set)
                zrow = ev[D:D + 1, :]
                nc.scalar.activation(zrow, zrow, Act.Ln)
                zinv = z_pool.tile([1, S], bf16, tag="z", name=f"zinv{h}")
                nc.scalar.activation(zinv, zrow, Act.Exp, scale=-1.0)
                for g in range(NQG):
                    bps = psB.tile([D, QG], f32, tag="b")
                    nc.tensor.matmul(
                        bps,
                        lhsT=ones64[:, :],
                        rhs=zinv[:, g * QG:(g + 1) * QG],
                        start=True, stop=True)
                    nc.vector.tensor_mul(
                        attnT[h][:, g * QG:(g + 1) * QG],
                        ev[0:D, g * QG:(g + 1) * QG], bps)

        # ---------------- phase C: output projection ----------------
        with tc.tile_pool(name="outst", bufs=4) as out_pool, \
             tc.tile_pool(name="psO", bufs=4, space="PSUM") as psO:
            for st in range(NST):
                for eg in range(NEG):
                    ops = psO.tile([P, EGW], f32, tag="o")
                    for h in range(HL):
                        nc.tensor.matmul(
                            ops,
                            lhsT=attnT[h][:, st * P:(st + 1) * P],
                            rhs=wo_sb[h][:, eg * EGW:(eg + 1) * EGW],
                            start=(h == 0), stop=(h == HL - 1))
                    ot = out_pool.tile([P, EGW], f32, tag="ot")
                    nc.vector.tensor_copy(ot, ops)
                    nc.sync.dma_start(
                        out=d_out[st * P:(st + 1) * P, eg * EGW:(eg + 1) * EGW],
                        in_=ot)

    _split_multi_waits(nc)
    return nc


# ---------------------------------------------------------------------------
# Host entry point
# ---------------------------------------------------------------------------
LAST_EXEC_NS = None
LAST_RESULT = None


def kernel(query, key, value, mask, Wq, Wk, Wv, Wo, bo):
    global LAST_EXEC_NS, LAST_RESULT
    _install_tile_drain_patch()
    from concourse.bass_utils import run_bass_kernel_spmd

    B, S, E = 2, 2048, 1024
    H, D = 16, 64
    N_CORES = 8
    BG = 2                    # batch groups
    HG = N_CORES // BG        # head groups per batch
    HL = H // HG              # heads per core
    DIM = HL * D

    query = np.asarray(query, dtype=np.float32)
    key = np.asarray(key, dtype=np.float32)
    value = np.asarray(value, dtype=np.float32)
    mask2d = np.asarray(mask).reshape(S, S).astype(bool)
    Wq = np.asarray(Wq, dtype=np.float32)
    Wk = np.asarray(Wk, dtype=np.float32)
    Wv = np.asarray(Wv, dtype=np.float32)
    Wo = np.asarray(Wo, dtype=np.float32)
    bo = np.asarray(bo, dtype=np.float32)

    bias_idx, biases, block_live = classify_mask(mask2d, S)
    nuniq = len(biases)
    bias_stack = (np.concatenate(biases, axis=1) if nuniq
                  else np.zeros((128, 128), np.float32))

    nc = build_nc(S, E, D, HL, bias_idx, block_live, nuniq)

    scale = np.float32(1.0 / np.sqrt(D))
    in_maps = []
    for c in range(N_CORES):
        b, hg = c // HG, c % HG
        cols = slice(hg * DIM, (hg + 1) * DIM)
        wv_l = Wv[:, cols].reshape(E, HL, D)
        wv_aug = np.zeros((E, HL, D + 1), np.float32)
        wv_aug[:, :, :D] = wv_l
        in_maps.append({
            "xqT": _bf16(query[b].T),
            "xkT": _bf16(key[b].T),
            "xvT": _bf16(value[b].T),
            "wq": _bf16(Wq[:, cols] * scale),
            "wk": _bf16(Wk[:, cols]),
            "wv": _bf16(wv_aug.reshape(E, HL * (D + 1))),
            "wo": _bf16(Wo[cols, :]),
            "biasT": np.ascontiguousarray(bias_stack),
        })

    res = run_bass_kernel_spmd(nc, in_maps, list(range(N_CORES)))
    LAST_RESULT = res
    LAST_EXEC_NS = res.exec_time_ns or res.mean_exec_time_ns

    out = np.empty((B, S, E), np.float32)
    for b in range(BG):
        acc = res.results[b * HG]["out_p"].astype(np.float32)
        for j in range(1, HG):
            acc = acc + res.results[b * HG + j]["out_p"]
        out[b] = acc + bo[None, :]
    return out


def _bf16(a):
    import ml_dtypes
    return np.ascontiguousarray(np.asarray(a, np.float32)).astype(
        ml_dtypes.bfloat16)
